# revision 1
# baseline (speedup 1.0000x reference)
"""TRN2 Bass kernel for channel cross-attention (XCA-style).

Math (per batch element b, matching the jax reference):
  qp = Wq q + bq ; kp = Wk k + bk           (1x1 convs, q/k: (192, 16384))
  qn = qp / max(||qp||_row, eps) ; kn likewise (L2 norm over the 16384 axis)
  A  = softmax_d(qn_c . kn_d * temp_h)       per head (6 heads x 32 ch)
  out = Wo (A (Wv v + bv)) + bo

Kernel strategy (one batch element per NeuronCore, 8 cores):
  Pass 1 streams q,k once: a stacked conv ([qp^T | kp^T] per 128-px chunk via a
  block-diagonal weight rhs), accumulating the raw cross-gram G = qp kp^T and
  per-channel sum-of-squares in PSUM across all 128 chunks. Norms, temperature
  and softmax are then applied on the tiny (192, 32) per-head logit blocks.
  The v path is folded: W_comb = Wo A_blockdiag Wv and
  b_comb = Wo A bv + bo are computed on-chip (192x192), so pass 2 is a single
  1x1 conv streaming v once: out = W_comb v + b_comb.

Matmuls run in float32r (fast fp32 mode, ~1e-3 rel err) by default.
"""

import numpy as np
from contextlib import ExitStack

import concourse.bass as bass
import concourse.tile as tile
from concourse import bacc, mybir
from concourse.bass_utils import run_bass_kernel_spmd

DIM = 192
HEADS = 6
CH = 32
HW = 16384
B = 8
EPS = 1e-12

PXT = 1024                # pixels per DMA tile
# last tiles smaller: shortens the serial pass-1 compute tail
TILE_SIZES = [1024] * 15 + [512, 512]
TILE_OFFS = [sum(TILE_SIZES[:i]) for i in range(len(TILE_SIZES))]
NPXT = len(TILE_SIZES)
CHUNK = 128               # pixels per matmul chunk (K of gram)
MMN = 512                 # max matmul free size (fp32 PSUM bank)

F32 = mybir.dt.float32
F32R = mybir.dt.float32r

MM_DT = F32R              # matmul operand dtype (F32R fast / F32 exact)
GRAM_N = 256              # padded gram free size (>=256 for f32r full speed)

_CACHE = {}


def _build():
    nc = bacc.Bacc("TRN2", target_bir_lowering=False, debug=False)

    # q/k carry a host-prepended ones row (bias folded into the conv matmul)
    q_d = nc.declare_dram_parameter("q", [DIM + 1, HW], MM_DT, isOutput=False)
    k_d = nc.declare_dram_parameter("k", [DIM + 1, HW], MM_DT, isOutput=False)
    v_d = nc.declare_dram_parameter("v", [DIM, HW], MM_DT, isOutput=False)
    # all weights packed into one (128, 2048) param -> one DMA
    wpk_d = nc.declare_dram_parameter("wpack", [128, 2048], MM_DT, isOutput=False)
    out_d = nc.declare_dram_parameter("out", [DIM, HW], F32, isOutput=True)

    with tile.TileContext(nc) as tc, ExitStack() as ctx:
        wp = ctx.enter_context(tc.tile_pool(name="weights", bufs=1))
        pp = ctx.enter_context(tc.tile_pool(name="post", bufs=1))
        vpool = ctx.enter_context(tc.tile_pool(name="v_res", bufs=1))

        KB = DIM + 1 - 128  # 65: rows of the second conv K-chunk
        wpk = wp.tile([128, 2048], MM_DT, tag="wpk")
        # conv weights (cols 0:1024) land first; the rest can trail the
        # first input tiles in the DMA queue
        nc.sync.dma_start(wpk[:, 0:1024], wpk_d[:, 0:1024])
        wq0 = wpk[:, 0:256]
        wk0 = wpk[:, 256:512]
        wq1 = wpk[0:KB, 512:768]
        wk1 = wpk[0:KB, 768:1024]
        wv_t = wpk[:, 1024:1216]
        wv_b = wpk[0:64, 1216:1408]
        woT_t = wpk[:, 1408:1600]
        woT_b = wpk[0:64, 1600:1792]
        bv_t = wpk[:, 1792:1793]
        bv_b = wpk[0:64, 1793:1794]
        bo_r = wpk[0:1, 1794:1986].bitcast(F32)
        tmp_r = wpk[0:1, 1986:1992].bitcast(F32)

        ones_col = wp.tile([128, 2], MM_DT, tag="ones_col")
        nc.vector.memset(ones_col[:].bitcast(F32), 1.0)
        # preload ACT tables for Sqrt/Exp during pass-1 (the first use of an
        # activation set pays a ~1.3us table load; keep it off the post chain)
        warm = wp.tile([1, 2], F32, tag="warm")
        nc.vector.memset(warm[:], 1.0)
        warm2 = wp.tile([1, 2], F32, tag="warm2")
        nc.scalar.sqrt(warm2[:], warm[:])
        nc.scalar.activation(warm2[:], warm[:], mybir.ActivationFunctionType.Exp)
        # per-channel temperature row tile (filled at pass-1 iter 1, after
        # the wpack part-B DMA that carries `temp` has been emitted)
        temp192 = wp.tile([1, DIM], F32, tag="temp192")

        run_q = pp.tile([128, DIM], F32, tag="run_q")
        run_k = pp.tile([128, DIM], F32, tag="run_k")
        v_tiles = []

        with tc.tile_pool(name="acc_psum", bufs=1, space="PSUM") as accp:
            # long-lived PSUM accumulators (one bank each)
            g_top = accp.tile([128, GRAM_N], F32, tag="g_top")
            g_bot = accp.tile([64, GRAM_N], F32, tag="g_bot")

            # ---------------- pass 1: stream q, k (and prefetch v) ---------
            with tc.tile_pool(name="p1_in", bufs=2) as inp, \
                 tc.tile_pool(name="p1_mid", bufs=4) as midp, \
                 tc.tile_pool(name="p1_psum", bufs=3, space="PSUM") as convp:
                for i in range(NPXT):
                    SZ = TILE_SIZES[i]
                    off = TILE_OFFS[i]
                    px = slice(off, off + SZ)
                    tA = inp.tile([128, PXT], MM_DT, tag="tA")
                    tB = inp.tile([KB, PXT], MM_DT, tag="tB")
                    tC = inp.tile([128, PXT], MM_DT, tag="tC")
                    tD = inp.tile([KB, PXT], MM_DT, tag="tD")
                    nc.sync.dma_start(tA[:, 0:SZ], q_d[0:128, px])
                    nc.sync.dma_start(tB[:, 0:SZ], q_d[128:DIM + 1, px])
                    if i == 0:
                        nc.sync.dma_start(wpk[:, 1024:2048], wpk_d[:, 1024:2048])
                    nc.sync.dma_start(tC[:, 0:SZ], k_d[0:128, px])
                    nc.sync.dma_start(tD[:, 0:SZ], k_d[128:DIM + 1, px])
                    if i == 1:
                        # temp repeated 32x per head (in*0 + bias fill); off
                        # the critical path, after wpack part B exists
                        for h in range(HEADS):
                            nc.scalar.activation(
                                temp192[0:1, h * CH:(h + 1) * CH],
                                wpk[0:1, 0:CH].bitcast(F32),
                                mybir.ActivationFunctionType.Identity,
                                bias=tmp_r[0:1, h:h + 1], scale=0.0)
                    if i == NPXT - 1:
                        # warm the sqrt act-table set while pass 1 drains
                        # (Copy doesn't evict; keeps the load off the post chain)
                        nc.scalar.sqrt(warm2[:], warm[:])

                    for j in range(SZ // CHUNK):
                        first = (i == 0 and j == 0)
                        last = (i == NPXT - 1 and j == SZ // CHUNK - 1)
                        cs = slice(j * CHUNK, (j + 1) * CHUNK)
                        # conv: out = [ones;q]^T [b;W^T], cols 192:256 zero-pad
                        qps = convp.tile([128, GRAM_N], F32, tag="qps")
                        nc.tensor.matmul(qps[:], tA[:, cs], wq0,
                                         start=True, stop=False)
                        nc.tensor.matmul(qps[:], tB[:, cs], wq1,
                                         start=False, stop=True)
                        kps = convp.tile([128, GRAM_N], F32, tag="kps")
                        nc.tensor.matmul(kps[:], tC[:, cs], wk0,
                                         start=True, stop=False)
                        nc.tensor.matmul(kps[:], tD[:, cs], wk1,
                                         start=False, stop=True)

                        # qp^T / kp^T chunks to SBUF (f32r rounding via ACT)
                        qT = midp.tile([128, DIM], MM_DT, tag="qT")
                        nc.scalar.copy(qT[:], qps[:, 0:DIM])
                        kT = midp.tile([128, GRAM_N], MM_DT, tag="kT")
                        nc.scalar.copy(kT[:], kps[:])
                        sq_q = midp.tile([128, DIM], F32, tag="sq_q")
                        nc.vector.tensor_mul(sq_q[:], qT[:], qT[:])
                        sq_k = midp.tile([128, DIM], F32, tag="sq_k")
                        nc.vector.tensor_mul(sq_k[:], kT[:, 0:DIM], kT[:, 0:DIM])
                        # running sums of squares: two parallel serial chains
                        # (q on DVE, k on gpsimd) so neither paces the loop
                        if first:
                            nc.vector.tensor_copy(run_q[:], sq_q[:])
                            nc.gpsimd.tensor_copy(run_k[:], sq_k[:])
                        else:
                            nc.vector.tensor_add(run_q[:], run_q[:], sq_q[:])
                            nc.gpsimd.tensor_add(run_k[:], run_k[:], sq_k[:])

                        # raw cross-gram accumulation (N padded to GRAM_N;
                        # pad cols of kT are exact zeros)
                        nc.tensor.matmul(g_top[:], qT[:, 0:128], kT[:],
                                         start=first, stop=last)
                        nc.tensor.matmul(g_bot[:], qT[:, 128:DIM], kT[:],
                                         start=first, stop=last)

                    # prefetch v for pass 2 (keeps the DMA queue saturated;
                    # v stays SBUF-resident until consumed)
                    vt = vpool.tile([128, SZ], MM_DT, tag=f"vt{i}")
                    nc.sync.dma_start(vt[:], v_d[0:128, px])
                    vb = vpool.tile([64, SZ], MM_DT, tag=f"vb{i}")
                    nc.sync.dma_start(vb[:], v_d[128:192, px])
                    v_tiles.append((vt, vb))

            # ---- norms + column-scaled gram (reads PSUM accumulators) ----
            run_sq_r = pp.tile([128, 2 * DIM], MM_DT, tag="run_sq_r")
            nc.scalar.copy(run_sq_r[:, 0:DIM], run_q[:])
            nc.vector.tensor_copy(run_sq_r[:, DIM:], run_k[:])
            with tc.tile_pool(name="ssq_psum", bufs=1, space="PSUM") as sspp:
                ssq = sspp.tile([128, 4], F32, tag="ssq")
                nc.tensor.matmul(ssq[:, 0:2], run_sq_r[:, 0:128], ones_col[:],
                                 start=True, stop=False)
                nc.tensor.matmul(ssq[0:64, 2:4], run_sq_r[:, 128:192],
                                 ones_col[:], start=False, stop=True)
                # rk in row form: row-ssq_k via ones contraction (fp32r ok:
                # N=192 is even; 4cyc/row but one-time)
                ssqk_row = sspp.tile([2, DIM], F32, tag="ssqk_row")
                nc.tensor.matmul(ssqk_row[:], ones_col[:],
                                 run_sq_r[:, DIM:2 * DIM], start=True, stop=True)
                ssq_sb = pp.tile([128, 4], F32, tag="ssq_sb")
                nc.scalar.sqrt(ssq_sb[:], ssq[:])       # q norms
                norms = pp.tile([128, 4], F32, tag="norms")
                nc.vector.tensor_scalar_max(norms[:], ssq_sb[:], EPS)
                rsq = pp.tile([128, 4], F32, tag="rsq")
                nc.vector.reciprocal(rsq[:], norms[:])
                # rsq col pairs: 0=rq[0:128], 2=rq[128:192]
                nrm_row = pp.tile([1, DIM], F32, tag="rowA")
                nc.scalar.sqrt(nrm_row[:], ssqk_row[0:1, 0:DIM])
            # preload the exp act-set now; the DVE/Pool chain below hides it
            nc.scalar.activation(warm2[:], warm[:],
                                 mybir.ActivationFunctionType.Exp)
            nrm2_row = pp.tile([1, DIM], F32, tag="rowB")
            nc.vector.tensor_scalar_max(nrm2_row[:], nrm_row[:], EPS)
            rk_row = pp.tile([1, DIM], F32, tag="rowA")
            nc.vector.reciprocal(rk_row[:], nrm2_row[:])
            rk2 = pp.tile([1, DIM], F32, tag="rowB")
            nc.vector.tensor_mul(rk2[:], rk_row[:], temp192[:])
            Bt = pp.tile([128, DIM], F32, tag="Bt")
            nc.gpsimd.partition_broadcast(Bt[:], rk2[:])

            Gs_t = pp.tile([128, DIM], F32, tag="Gs_t")
            nc.vector.tensor_mul(Gs_t[:], g_top[:, 0:DIM], Bt[:])
            Gs_b = pp.tile([64, DIM], F32, tag="Gs_b")
            nc.vector.tensor_mul(Gs_b[:], g_bot[:, 0:DIM], Bt[0:64, :])
        # acc_psum closed here - PSUM free for the small matmuls below

        # ---- compact per-head logits + softmax (SBUF only) ----
        C1 = pp.tile([128, CH], F32, tag="C1")
        C2 = pp.tile([64, CH], F32, tag="C2")
        for h in range(4):
            hs = slice(h * CH, (h + 1) * CH)
            nc.scalar.mul(C1[hs, :], Gs_t[hs, hs], rsq[hs, 0:1])
        for h in range(4, HEADS):
            ps = slice((h - 4) * CH, (h - 3) * CH)
            hs = slice(h * CH, (h + 1) * CH)
            nc.scalar.mul(C2[ps, :], Gs_b[ps, hs], rsq[ps, 2:3])

        E1 = pp.tile([128, CH], F32, tag="E1")
        den1 = pp.tile([128, 1], F32, tag="den1")
        nc.scalar.activation(E1[:], C1[:], mybir.ActivationFunctionType.Exp,
                             accum_out=den1[:])
        E2 = pp.tile([64, CH], F32, tag="E2")
        den2 = pp.tile([64, 1], F32, tag="den2")
        nc.scalar.activation(E2[:], C2[:], mybir.ActivationFunctionType.Exp,
                             accum_out=den2[:])
        rden1 = pp.tile([128, 1], F32, tag="rden1")
        nc.vector.reciprocal(rden1[:], den1[:])
        rden2 = pp.tile([64, 1], F32, tag="rden2")
        nc.vector.reciprocal(rden2[:], den2[:])

        # block-diagonal attention matrix A (rows scaled by 1/den)
        BD_t = pp.tile([128, DIM], F32, tag="BD_t")
        nc.vector.memset(BD_t[:], 0.0)
        BD_b = pp.tile([64, DIM], F32, tag="BD_b")
        nc.vector.memset(BD_b[:], 0.0)
        for h in range(4):
            hs = slice(h * CH, (h + 1) * CH)
            nc.scalar.mul(BD_t[hs, hs], E1[hs, :], rden1[hs, 0:1])
        for h in range(4, HEADS):
            ps = slice((h - 4) * CH, (h - 3) * CH)
            hs = slice(h * CH, (h + 1) * CH)
            nc.scalar.mul(BD_b[ps, hs], E2[ps, :], rden2[ps, 0:1])

        # ---- X1 = A^T Wo^T ; W_comb^T = Wv^T X1 ; b_row = bv^T X1 ----
        with tc.tile_pool(name="post_psum", bufs=1, space="PSUM") as ppp:
            X1t = ppp.tile([128, DIM], F32, tag="X1t")
            X1b = ppp.tile([64, DIM], F32, tag="X1b")
            nc.tensor.matmul(X1t[:], BD_t[:, 0:128], woT_t.bitcast(F32),
                             start=True, stop=False)
            nc.tensor.matmul(X1t[:], BD_b[:, 0:128], woT_b.bitcast(F32),
                             start=False, stop=True)
            nc.tensor.matmul(X1b[:], BD_t[:, 128:DIM], woT_t.bitcast(F32),
                             start=True, stop=False)
            nc.tensor.matmul(X1b[:], BD_b[:, 128:DIM], woT_b.bitcast(F32),
                             start=False, stop=True)
            X1t_sb = pp.tile([128, DIM], F32, tag="X1t_sb")
            nc.scalar.copy(X1t_sb[:], X1t[:])
            X1b_sb = pp.tile([64, DIM], F32, tag="X1b_sb")
            nc.scalar.copy(X1b_sb[:], X1b[:])

            # brow first: its bias-transpose DMAs then overlap the P matmuls
            brow = ppp.tile([1, DIM], F32, tag="brow")
            nc.tensor.matmul(brow[:], bv_t.bitcast(F32), X1t_sb[:], start=True, stop=False)
            nc.tensor.matmul(brow[:], bv_b.bitcast(F32), X1b_sb[:], start=False, stop=True)
            bc_row = pp.tile([1, DIM], F32, tag="bc_row")
            nc.vector.tensor_add(bc_row[:], brow[:], bo_r)
            bc_t = pp.tile([128, 1], F32, tag="bc_t")
            nc.sync.dma_start(bc_t[:], bc_row[0:1, 0:128])
            bc_b = pp.tile([64, 1], F32, tag="bc_b")
            nc.sync.dma_start(bc_b[:], bc_row[0:1, 128:192])

            Pt = ppp.tile([128, DIM], F32, tag="Pt")
            Pb = ppp.tile([64, DIM], F32, tag="Pb")
            nc.tensor.matmul(Pt[:], wv_t[:, 0:128].bitcast(F32), X1t_sb[:],
                             start=True, stop=False)
            nc.tensor.matmul(Pt[:], wv_b[:, 0:128].bitcast(F32), X1b_sb[:],
                             start=False, stop=True)
            nc.tensor.matmul(Pb[:], wv_t[:, 128:DIM].bitcast(F32), X1t_sb[:],
                             start=True, stop=False)
            nc.tensor.matmul(Pb[:], wv_b[:, 128:DIM].bitcast(F32), X1b_sb[:],
                             start=False, stop=True)

            wcT_t = pp.tile([128, DIM], MM_DT, tag="wcT_t")
            nc.scalar.copy(wcT_t[:], Pt[:])
            wcT_b = pp.tile([64, DIM], MM_DT, tag="wcT_b")
            nc.scalar.copy(wcT_b[:], Pb[:])

        # ---------------- pass 2: out = W_comb v + b_comb ----------------
        with tc.tile_pool(name="p2_out", bufs=3) as op_, \
             tc.tile_pool(name="p2_psum", bufs=3, space="PSUM") as opp:
            for i in [NPXT - 1, NPXT - 2] + list(range(NPXT - 2)):
                SZ = TILE_SIZES[i]
                off = TILE_OFFS[i]
                px = slice(off, off + SZ)
                vt, vb = v_tiles[i]
                os_t = op_.tile([128, SZ], F32, tag="os_t")
                os_b = op_.tile([64, SZ], F32, tag="os_b")
                for h in range(SZ // MMN):
                    ms = slice(h * MMN, (h + 1) * MMN)
                    o_t = opp.tile([128, MMN], F32, tag="o_t")
                    o_b = opp.tile([64, MMN], F32, tag="o_b")
                    nc.tensor.matmul(o_t[:], wcT_t[:, 0:128], vt[:, ms],
                                     start=True, stop=False)
                    nc.tensor.matmul(o_t[:], wcT_b[:, 0:128], vb[:, ms],
                                     start=False, stop=True)
                    nc.tensor.matmul(o_b[:], wcT_t[:, 128:DIM], vt[:, ms],
                                     start=True, stop=False)
                    nc.tensor.matmul(o_b[:], wcT_b[:, 128:DIM], vb[:, ms],
                                     start=False, stop=True)
                    nc.scalar.activation(os_t[:, ms], o_t[:],
                                         mybir.ActivationFunctionType.Identity,
                                         bias=bc_t[:])
                    nc.vector.tensor_scalar_add(os_b[:, ms], o_b[:], bc_b[:])
                nc.sync.dma_start(out_d[0:128, px], os_t[:, 0:SZ])
                nc.sync.dma_start(out_d[128:192, px], os_b[:, 0:SZ])

    nc.compile()
    return nc


def _get_nc():
    if "nc" not in _CACHE:
        _CACHE["nc"] = _build()
    return _CACHE["nc"]


def _make_in_maps(inputs):
    q = np.asarray(inputs["q"], dtype=np.float32)
    k = np.asarray(inputs["k"], dtype=np.float32)
    v = np.asarray(inputs["v"], dtype=np.float32)
    wq = np.asarray(inputs["wq"], dtype=np.float32)
    wk = np.asarray(inputs["wk"], dtype=np.float32)
    wv_ = np.asarray(inputs["wv"], dtype=np.float32)
    wo = np.asarray(inputs["wo"], dtype=np.float32)
    bq = np.asarray(inputs["bq"], dtype=np.float32)
    bk = np.asarray(inputs["bk"], dtype=np.float32)
    bv_ = np.asarray(inputs["bv"], dtype=np.float32)
    bo = np.asarray(inputs["bo"], dtype=np.float32)
    temp = np.asarray(inputs["temperature"], dtype=np.float32).reshape(1, HEADS)

    # conv rhs = [bias; W^T] (193, 192) split at row 128, cols padded to 256
    def conv_rhs(w, bias):
        aug = np.concatenate([bias.reshape(1, DIM), w.T], axis=0)  # (193, 192)
        pad = np.zeros((DIM + 1, 256), dtype=np.float32)
        pad[:, 0:DIM] = aug
        return pad[0:128], pad[128:]

    wq0, wq1 = conv_rhs(wq, bq)
    wk0, wk1 = conv_rhs(wk, bk)
    ones_row = np.ones((1, HW), dtype=np.float32)

    wpack = np.zeros((128, 2048), dtype=np.float32)
    wpack[:, 0:256] = wq0
    wpack[:, 256:512] = wk0
    wpack[0:65, 512:768] = wq1
    wpack[0:65, 768:1024] = wk1
    wpack[:, 1024:1216] = wv_[0:128]
    wpack[0:64, 1216:1408] = wv_[128:192]
    woT = wo.T
    wpack[:, 1408:1600] = woT[0:128]
    wpack[0:64, 1600:1792] = woT[128:192]
    wpack[:, 1792] = bv_[0:128]
    wpack[0:64, 1793] = bv_[128:192]
    wpack[0, 1794:1986] = bo
    wpack[0, 1986:1992] = temp.reshape(HEADS)

    shared = {"wpack": np.ascontiguousarray(wpack)}
    in_maps = []
    for b in range(B):
        m = dict(shared)
        m["q"] = np.ascontiguousarray(
            np.concatenate([ones_row, q[b].reshape(DIM, HW)], axis=0))
        m["k"] = np.ascontiguousarray(
            np.concatenate([ones_row, k[b].reshape(DIM, HW)], axis=0))
        m["v"] = np.ascontiguousarray(v[b].reshape(DIM, HW))
        in_maps.append(m)
    return in_maps


def _get_runner():
    """Compile once and cache a sharded-jit runner (run_bass_kernel_spmd
    rebuilds its jit closure per call, which re-traces every time)."""
    if "runner" in _CACHE:
        return _CACHE["runner"]
    import jax
    import jax.numpy as jnp
    from jax.sharding import Mesh, PartitionSpec
    from jax.experimental.shard_map import shard_map
    from concourse import bass2jax, mybir as mb
    from concourse.bass2jax import _bass_exec_p, partition_id_tensor

    bass2jax.install_neuronx_cc_hook()
    nc = _get_nc()

    partition_name = nc.partition_id_tensor.name if nc.partition_id_tensor else None
    in_names, out_names, out_avals = [], [], []
    for alloc in nc.m.functions[0].allocations:
        if not isinstance(alloc, mb.MemoryLocationSet):
            continue
        name = alloc.memorylocations[0].name
        if alloc.kind == "ExternalInput":
            if name != partition_name:
                in_names.append(name)
        elif alloc.kind == "ExternalOutput":
            out_names.append(name)
            out_avals.append(jax.core.ShapedArray(
                tuple(alloc.tensor_shape), mb.dt.np(alloc.dtype)))
    n_params = len(in_names)
    n_outs = len(out_avals)
    all_in_names = tuple(in_names + out_names +
                         ([partition_name] if partition_name else []))

    def _body(*args):
        operands = list(args)
        if partition_name is not None:
            operands.append(partition_id_tensor())
        return tuple(_bass_exec_p.bind(
            *operands,
            out_avals=tuple(out_avals),
            in_names=all_in_names,
            out_names=tuple(out_names),
            lowering_input_output_aliases=(),
            sim_require_finite=True,
            sim_require_nnan=True,
            nc=nc,
        ))

    devices = jax.devices()[:B]
    mesh = Mesh(np.asarray(devices), ("core",))
    in_specs = (PartitionSpec("core"),) * (n_params + n_outs)
    out_specs = (PartitionSpec("core"),) * n_outs
    donate = tuple(range(n_params, n_params + n_outs))
    sharded = jax.jit(
        shard_map(_body, mesh=mesh, in_specs=in_specs, out_specs=out_specs,
                  check_rep=False),
        donate_argnums=donate, keep_unused=True)

    zero_shapes = [(B * a.shape[0], *a.shape[1:]) for a in out_avals]
    zero_dtypes = [a.dtype for a in out_avals]

    def run(in_maps):
        concat_in = [
            np.concatenate([np.asarray(in_maps[c][nm]) for c in range(B)], axis=0)
            for nm in in_names
        ]
        zeros = [jnp.zeros(s, d) for s, d in zip(zero_shapes, zero_dtypes)]
        outs = sharded(*concat_in, *zeros)
        return {
            nm: np.asarray(outs[i]).reshape(B, *out_avals[i].shape)
            for i, nm in enumerate(out_names)
        }

    _CACHE["runner"] = run
    return run


def _prebuild():
    """Compile the NEFF and warm the jit at import time so the first real
    kernel() call doesn't pay the ~10s build; never let this break import."""
    try:
        run = _get_runner()
        z = np.zeros((DIM + 1, HW), dtype=np.float32)
        zv = np.zeros((DIM, HW), dtype=np.float32)
        zw = np.zeros((128, 2048), dtype=np.float32)
        run([{"q": z, "k": z, "v": zv, "wpack": zw} for _ in range(B)])
    except Exception:
        _CACHE.clear()


def kernel(q, k, v, wq, bq, wk, bk, wv, bv, wo, bo, temperature):
    run = _get_runner()
    in_maps = _make_in_maps(dict(q=q, k=k, v=v, wq=wq, bq=bq, wk=wk, bk=bk,
                                 wv=wv, bv=bv, wo=wo, bo=bo,
                                 temperature=temperature))
    out = run(in_maps)["out"].reshape(B, DIM, 128, 128)
    return np.ascontiguousarray(out.astype(np.float32))


import os as _os
if not _os.environ.get("KERNEL_NO_PREBUILD"):
    _prebuild()



# revision 6
# speedup vs baseline: 2.4755x; 2.4755x over previous
"""TRN2 Bass kernel for channel cross-attention (XCA-style).

Math (per batch element b, matching the jax reference):
  qp = Wq q + bq ; kp = Wk k + bk           (1x1 convs, q/k: (192, 16384))
  qn = qp / max(||qp||_row, eps) ; kn likewise (L2 norm over the 16384 axis)
  A  = softmax_d(qn_c . kn_d * temp_h)       per head (6 heads x 32 ch)
  out = Wo (A (Wv v + bv)) + bo

Strategy (one batch element per core, 8 cores) — DMA-traffic-minimal:
  Pass 1 streams q,k ONCE as fp8(e4m3), pixel-major, with a ones channel
  prepended (aq = [1; q], 193 ch).  Raw grams are accumulated in PSUM with
  fp8 DoubleRow matmuls (2 pixels per partition, 2x rate):
      Gqq = aq aq^T,  Gqk = aq ak^T,  Gkk = ak ak^T     (193x193 each)
  Everything attention-related derives from these on-chip:
      row norms   ||qp_c||^2 = diag(Wq'^T Gqq Wq'),  Wq' = [bq; Wq^T]
      logits      P = Wq'^T Gqk Wk'
  (fp8 is plenty here: logits are tiny (~1e-2) and softmax deviations
  contribute <1% of the output, so gram noise is damped ~100x.)
  The v path stays bf16 end-to-end: softmax -> block-diag A -> fold
  W_comb^T = (Wo A Wv)^T and b_comb on-chip; pass 2 is a single conv
  out^T = [v;1]^T [Wc^T; bc], streamed per 128-pixel chunk (M=pixels),
  written to DRAM as bf16 pixel-major (host transposes back).

  DMA totals ~19 MB/core (fp8 q,k + bf16 v,out) vs ~51 MB for f32.
"""

import numpy as np
from contextlib import ExitStack

import concourse.bass as bass
import concourse.tile as tile
from concourse import bacc, mybir

DIM = 192
HEADS = 6
CH = 32
HW = 16384
B = 8
C1 = DIM + 1              # aug channels (ones row first)

NT = 8                    # pass-1 tiles per input
PXT = HW // NT            # 2048 pixels per pass-1 tile
GPT = PXT // 256          # 8 DoubleRow groups (256 px) per tile
W1 = 16 * C1              # 3088 fp8 bytes per partition per pass-1 tile

NG2 = HW // 256           # 64 output groups (256 px)
GPS = 4                   # groups per output staging tile (1024 px)
NST = NG2 // GPS          # 16 staging tiles / output DMAs

F32 = mybir.dt.float32
BF16 = mybir.dt.bfloat16
F8 = mybir.dt.float8e4
DR = mybir.MatmulPerfMode.DoubleRow
IDENT = mybir.ActivationFunctionType.Identity
EXP = mybir.ActivationFunctionType.Exp

USE_DR = True             # fp8 DoubleRow (2x PE) for the pass-1 grams

_CACHE = {}


def _build():
    nc = bacc.Bacc("TRN2", target_bir_lowering=False, debug=False)

    # pixel-major fp8 aug inputs: row r holds pixels 16r..16r+15, each 193ch
    q_d = nc.declare_dram_parameter("q8", [HW // 16, W1], F8, isOutput=False)
    k_d = nc.declare_dram_parameter("k8", [HW // 16, W1], F8, isOutput=False)
    # v: channel-major bf16, pixels permuted [even|odd] per 256-block, plus
    # a ones row (193) so pass 2's bias rides the matmul
    v_d = nc.declare_dram_parameter("vb", [C1, HW], BF16, isOutput=False)
    # bf16 weights pack: cols [Wq' | Wk' | Wv | Wo^T | bv]
    wb_d = nc.declare_dram_parameter("wb", [C1, 772], BF16, isOutput=False)
    # f32 smalls: [bo(192) | temp(6) | pad]
    wf_d = nc.declare_dram_parameter("wf", [1, 200], F32, isOutput=False)
    # out^T, pixel-major bf16: (tile, group, partition, 2x192)
    out_d = nc.declare_dram_parameter("out", [NST, GPS, 128, 384], BF16,
                                      isOutput=True)

    with tile.TileContext(nc) as tc, ExitStack() as ctx:
        wp = ctx.enter_context(tc.tile_pool(name="weights", bufs=1))
        pp = ctx.enter_context(tc.tile_pool(name="post", bufs=1))
        vpool = ctx.enter_context(tc.tile_pool(name="v_res", bufs=1))

        wb_t = wp.tile([128, 772], BF16, tag="wb_t")
        wb_b = wp.tile([65, 772], BF16, tag="wb_b")
        nc.sync.dma_start(wb_t[:], wb_d[0:128, :])
        nc.sync.dma_start(wb_b[:], wb_d[128:C1, :])
        wf_t = wp.tile([1, 200], F32, tag="wf")
        nc.sync.dma_start(wf_t[:], wf_d[:])

        wq_t = wb_t[:, 0:192]          # Wq' rows 0:128   [128, 192]
        wq_b = wb_b[:, 0:192]          # Wq' rows 128:193 [65, 192]
        wk_t = wb_t[:, 192:384]
        wk_b = wb_b[:, 192:384]
        wv_t = wb_t[:, 384:576]        # Wv rows 0:128
        wv_b = wb_b[0:64, 384:576]     # Wv rows 128:192
        wo_t = wb_t[:, 576:768]        # Wo^T rows 0:128
        wo_b = wb_b[0:64, 576:768]
        bv_t = wb_t[:, 768:769]
        bv_b = wb_b[0:64, 768:769]
        bo_r = wf_t[0:1, 0:192]
        temp6 = wf_t[0:1, 192:198]

        ones128 = wp.tile([128, 2], BF16, tag="ones128")
        nc.vector.memset(ones128[:], 1.0)
        ones65 = wp.tile([65, 2], BF16, tag="ones65")
        nc.vector.memset(ones65[:], 1.0)
        temp192 = wp.tile([1, DIM], F32, tag="temp192")

        v_tiles = []

        # ---------------- pass 1: fp8 raw grams ----------------
        with tc.tile_pool(name="acc_psum", bufs=1, space="PSUM") as accp:
            gqq_t = accp.tile([128, C1], F32, tag="gqq_t")
            gqk_t = accp.tile([128, C1], F32, tag="gqk_t")
            gkk_t = accp.tile([128, C1], F32, tag="gkk_t")
            gqq_b = accp.tile([65, C1], F32, tag="gqq_b")
            gqk_b = accp.tile([65, C1], F32, tag="gqk_b")
            gkk_b = accp.tile([65, C1], F32, tag="gkk_b")

            with tc.tile_pool(name="p1_in", bufs=2) as inp:
                for i in range(NT):
                    qt = inp.tile([128, W1], F8, tag="qt")
                    kt = inp.tile([128, W1], F8, tag="kt")
                    nc.sync.dma_start(qt[:], q_d[i * 128:(i + 1) * 128, :])
                    nc.sync.dma_start(kt[:], k_d[i * 128:(i + 1) * 128, :])
                    if i == 0:
                        # temp repeated 32x per head (in*0 + bias fill)
                        for h in range(HEADS):
                            nc.scalar.activation(
                                temp192[0:1, h * CH:(h + 1) * CH],
                                wf_t[0:1, 0:CH], IDENT,
                                bias=temp6[0:1, h:h + 1], scale=0.0)
                    for g in range(GPT):
                        first = (i == 0 and g == 0)
                        last = (i == NT - 1 and g == GPT - 1)
                        qg = qt[:, g * 2 * C1:(g + 1) * 2 * C1]
                        kg = kt[:, g * 2 * C1:(g + 1) * 2 * C1]
                        if USE_DR:
                            qs = qg.rearrange("p (two c) -> p two c", two=2)
                            ks = kg.rearrange("p (two c) -> p two c", two=2)
                            mm = [
                                (gqq_t, qs[:, :, 0:128], qs),
                                (gqk_t, qs[:, :, 0:128], ks),
                                (gkk_t, ks[:, :, 0:128], ks),
                                (gqq_b, qs[:, :, 128:C1], qs),
                                (gqk_b, qs[:, :, 128:C1], ks),
                                (gkk_b, ks[:, :, 128:C1], ks),
                            ]
                            for out, lhsT, rhs in mm:
                                nc.tensor.matmul(out[:], lhsT, rhs,
                                                 start=first, stop=last,
                                                 perf_mode=DR)
                        else:
                            for j in range(2):
                                qs = qg[:, j * C1:(j + 1) * C1]
                                ks = kg[:, j * C1:(j + 1) * C1]
                                st = first and j == 0
                                sp = last and j == 1
                                mm = [
                                    (gqq_t, qs[:, 0:128], qs),
                                    (gqk_t, qs[:, 0:128], ks),
                                    (gkk_t, ks[:, 0:128], ks),
                                    (gqq_b, qs[:, 128:C1], qs),
                                    (gqk_b, qs[:, 128:C1], ks),
                                    (gkk_b, ks[:, 128:C1], ks),
                                ]
                                for out, lhsT, rhs in mm:
                                    nc.tensor.matmul(out[:], lhsT, rhs,
                                                     start=st, stop=sp)
            # v prefetch: all AFTER q/k in the DMA queue, so pass-1 PE
            # (paced by q/k) never waits and the DMA engine never idles
            for i in range(NT):
                vt = vpool.tile([128, PXT], BF16, tag=f"vt{i}")
                nc.sync.dma_start(vt[:], v_d[0:128, i * PXT:(i + 1) * PXT])
                vbt = vpool.tile([65, PXT], BF16, tag=f"vb{i}")
                nc.sync.dma_start(vbt[:], v_d[128:C1, i * PXT:(i + 1) * PXT])
                v_tiles.append((vt, vbt))

            # gram copies PSUM -> SBUF bf16 (split ACT / DVE)
            gqq_st = pp.tile([128, C1], BF16, tag="gqq_st")
            nc.scalar.copy(gqq_st[:], gqq_t[:])
            gqk_st = pp.tile([128, C1], BF16, tag="gqk_st")
            nc.scalar.copy(gqk_st[:], gqk_t[:])
            gkk_st = pp.tile([128, C1], BF16, tag="gkk_st")
            nc.scalar.copy(gkk_st[:], gkk_t[:])
            gqq_sb = pp.tile([65, C1], BF16, tag="gqq_sb")
            nc.vector.tensor_copy(gqq_sb[:], gqq_b[:])
            gqk_sb = pp.tile([65, C1], BF16, tag="gqk_sb")
            nc.vector.tensor_copy(gqk_sb[:], gqk_b[:])
            gkk_sb = pp.tile([65, C1], BF16, tag="gkk_sb")
            nc.vector.tensor_copy(gkk_sb[:], gkk_b[:])
        # gram PSUM banks free from here

        # ---------------- norms: dq = diag(Wq'^T Gqq Wq') ----------------
        # (PSUM is 8 banks; post tiles are spread over three sequential pools)
        with tc.tile_pool(name="nrm_psum", bufs=1, space="PSUM") as npp:
            tq_t = npp.tile([128, DIM], F32, tag="tq_t")
            tq_b = npp.tile([65, DIM], F32, tag="tq_b")
            tk_t = npp.tile([128, DIM], F32, tag="tk_t")
            tk_b = npp.tile([65, DIM], F32, tag="tk_b")
            # Tq = Gqq @ Wq' (Gqq symmetric -> lhsT as stored)
            nc.tensor.matmul(tq_t[:], gqq_st[:, 0:128], wq_t,
                             start=True, stop=False)
            nc.tensor.matmul(tq_t[:], gqq_sb[:, 0:128], wq_b,
                             start=False, stop=True)
            nc.tensor.matmul(tq_b[:], gqq_st[:, 128:C1], wq_t,
                             start=True, stop=False)
            nc.tensor.matmul(tq_b[:], gqq_sb[:, 128:C1], wq_b,
                             start=False, stop=True)
            nc.tensor.matmul(tk_t[:], gkk_st[:, 0:128], wk_t,
                             start=True, stop=False)
            nc.tensor.matmul(tk_t[:], gkk_sb[:, 0:128], wk_b,
                             start=False, stop=True)
            nc.tensor.matmul(tk_b[:], gkk_st[:, 128:C1], wk_t,
                             start=True, stop=False)
            nc.tensor.matmul(tk_b[:], gkk_sb[:, 128:C1], wk_b,
                             start=False, stop=True)

            # pq = Wq' .* Tq  (bf16), dq_c = sum_i pq[i, c]
            pq_t = pp.tile([128, DIM], BF16, tag="pq_t")
            nc.vector.tensor_mul(pq_t[:], wq_t, tq_t[:])
            pq_b = pp.tile([65, DIM], BF16, tag="pq_b")
            nc.vector.tensor_mul(pq_b[:], wq_b, tq_b[:])
            pk_t = pp.tile([128, DIM], BF16, tag="pk_t")
            nc.vector.tensor_mul(pk_t[:], wk_t, tk_t[:])
            pk_b = pp.tile([65, DIM], BF16, tag="pk_b")
            nc.vector.tensor_mul(pk_b[:], wk_b, tk_b[:])

            # ST = Gqk^T-contraction step: ST[j, c] = sum_i Gqk[i,j] Wq'[i,c]
            st_t = npp.tile([128, DIM], F32, tag="st_t")
            st_b = npp.tile([65, DIM], F32, tag="st_b")
            nc.tensor.matmul(st_t[:], gqk_st[:, 0:128], wq_t,
                             start=True, stop=False)
            nc.tensor.matmul(st_t[:], gqk_sb[:, 0:128], wq_b,
                             start=False, stop=True)
            nc.tensor.matmul(st_b[:], gqk_st[:, 128:C1], wq_t,
                             start=True, stop=False)
            nc.tensor.matmul(st_b[:], gqk_sb[:, 128:C1], wq_b,
                             start=False, stop=True)

            st_st = pp.tile([128, DIM], BF16, tag="st_st")
            nc.scalar.copy(st_st[:], st_t[:])
            st_sb = pp.tile([65, DIM], BF16, tag="st_sb")
            nc.vector.tensor_copy(st_sb[:], st_b[:])

        with tc.tile_pool(name="dqp_psum", bufs=1, space="PSUM") as dpp:
            # dq as a COLUMN (per-partition scale for the head extraction):
            # dq[c, 0] = sum_i pq[i, c] via rhs=ones
            dq_t = dpp.tile([128, 2], F32, tag="dq_t")
            dq_b = dpp.tile([64, 2], F32, tag="dq_b")
            nc.tensor.matmul(dq_t[:, 0:2], pq_t[:, 0:128], ones128[:],
                             start=True, stop=False)
            nc.tensor.matmul(dq_t[:, 0:2], pq_b[:, 0:128], ones65[:],
                             start=False, stop=True)
            nc.tensor.matmul(dq_b[:, 0:2], pq_t[:, 128:DIM], ones128[:],
                             start=True, stop=False)
            nc.tensor.matmul(dq_b[:, 0:2], pq_b[:, 128:DIM], ones65[:],
                             start=False, stop=True)
            # dk as a ROW (free-dim scale): dk[0, d] = sum_i pk[i, d]
            dk_r = dpp.tile([2, DIM], F32, tag="dk_r")
            nc.tensor.matmul(dk_r[:], ones128[:], pk_t[:],
                             start=True, stop=False)
            nc.tensor.matmul(dk_r[:], ones65[:], pk_b[:],
                             start=False, stop=True)

            # P = Wq'^T Gqk Wk' : P[c, d] = sum_j ST[j, c] Wk'[j, d]
            p_t = dpp.tile([128, DIM], F32, tag="p_t")
            p_b = dpp.tile([64, DIM], F32, tag="p_b")
            nc.tensor.matmul(p_t[:], st_st[:, 0:128], wk_t,
                             start=True, stop=False)
            nc.tensor.matmul(p_t[:], st_sb[:, 0:128], wk_b,
                             start=False, stop=True)
            nc.tensor.matmul(p_b[:], st_st[:, 128:DIM], wk_t,
                             start=True, stop=False)
            nc.tensor.matmul(p_b[:], st_sb[:, 128:DIM], wk_b,
                             start=False, stop=True)

            # rq = rsqrt(max(dq, tiny)) per-partition; rk row likewise
            mq_t = pp.tile([128, 1], F32, tag="mq_t")
            nc.vector.tensor_scalar_max(mq_t[:], dq_t[:, 0:1], 1e-20)
            mq_b = pp.tile([64, 1], F32, tag="mq_b")
            nc.vector.tensor_scalar_max(mq_b[:], dq_b[:, 0:1], 1e-20)
            mk_r = pp.tile([1, DIM], F32, tag="mk_r")
            nc.vector.tensor_scalar_max(mk_r[:], dk_r[0:1, :], 1e-20)
            sq_t = pp.tile([128, 1], F32, tag="sq_t")
            nc.scalar.sqrt(sq_t[:], mq_t[:])
            sq_b = pp.tile([64, 1], F32, tag="sq_b")
            nc.scalar.sqrt(sq_b[:], mq_b[:])
            sk_r = pp.tile([1, DIM], F32, tag="sk_r")
            nc.scalar.sqrt(sk_r[:], mk_r[:])
            rq_t = pp.tile([128, 1], F32, tag="rq_t")
            nc.vector.reciprocal(rq_t[:], sq_t[:])
            rq_b = pp.tile([64, 1], F32, tag="rq_b")
            nc.vector.reciprocal(rq_b[:], sq_b[:])
            rk_r = pp.tile([1, DIM], F32, tag="rk_r")
            nc.vector.reciprocal(rk_r[:], sk_r[:])
            rkt = pp.tile([1, DIM], F32, tag="rkt")
            nc.vector.tensor_mul(rkt[:], rk_r[:], temp192[:])
            Bt = pp.tile([128, DIM], F32, tag="Bt")
            nc.gpsimd.partition_broadcast(Bt[:], rkt[:])

            # scale the k side (rk * temp) while still full-width
            ps_t = pp.tile([128, DIM], F32, tag="ps_t")
            nc.vector.tensor_mul(ps_t[:], p_t[:], Bt[:])
            ps_b = pp.tile([64, DIM], F32, tag="ps_b")
            nc.vector.tensor_mul(ps_b[:], p_b[:], Bt[0:64, :])

        with tc.tile_pool(name="p_psum", bufs=1, space="PSUM") as ppp:
            # per-head block extraction with the q-side scale -> logits
            c1 = pp.tile([128, CH], F32, tag="c1")
            c2 = pp.tile([64, CH], F32, tag="c2")
            for h in range(4):
                hs = slice(h * CH, (h + 1) * CH)
                nc.scalar.mul(c1[hs, :], ps_t[hs, hs], rq_t[hs, 0:1])
            for h in range(4, HEADS):
                ps = slice((h - 4) * CH, (h - 3) * CH)
                hs = slice(h * CH, (h + 1) * CH)
                nc.scalar.mul(c2[ps, :], ps_b[ps, hs], rq_b[ps, 0:1])

            # softmax over the free dim (per 32-wide head block)
            e1 = pp.tile([128, CH], F32, tag="e1")
            den1 = pp.tile([128, 1], F32, tag="den1")
            nc.scalar.activation(e1[:], c1[:], EXP, accum_out=den1[:])
            e2 = pp.tile([64, CH], F32, tag="e2")
            den2 = pp.tile([64, 1], F32, tag="den2")
            nc.scalar.activation(e2[:], c2[:], EXP, accum_out=den2[:])
            rden1 = pp.tile([128, 1], F32, tag="rden1")
            nc.vector.reciprocal(rden1[:], den1[:])
            rden2 = pp.tile([64, 1], F32, tag="rden2")
            nc.vector.reciprocal(rden2[:], den2[:])

            bd_t = pp.tile([128, DIM], BF16, tag="bd_t")
            nc.vector.memset(bd_t[:], 0.0)
            bd_b = pp.tile([64, DIM], BF16, tag="bd_b")
            nc.vector.memset(bd_b[:], 0.0)
            for h in range(4):
                hs = slice(h * CH, (h + 1) * CH)
                nc.scalar.mul(bd_t[hs, hs], e1[hs, :], rden1[hs, 0:1])
            for h in range(4, HEADS):
                ps = slice((h - 4) * CH, (h - 3) * CH)
                hs = slice(h * CH, (h + 1) * CH)
                nc.scalar.mul(bd_b[ps, hs], e2[ps, :], rden2[ps, 0:1])

            # ---- folds: X = (Wo A)^T ; Wc^T = Wv^T-contract X ; bc ----
            x_t = ppp.tile([128, DIM], F32, tag="x_t")
            x_b = ppp.tile([64, DIM], F32, tag="x_b")
            nc.tensor.matmul(x_t[:], bd_t[:, 0:128], wo_t,
                             start=True, stop=False)
            nc.tensor.matmul(x_t[:], bd_b[:, 0:128], wo_b,
                             start=False, stop=True)
            nc.tensor.matmul(x_b[:], bd_t[:, 128:DIM], wo_t,
                             start=True, stop=False)
            nc.tensor.matmul(x_b[:], bd_b[:, 128:DIM], wo_b,
                             start=False, stop=True)
            x_st = pp.tile([128, DIM], BF16, tag="x_st")
            nc.scalar.copy(x_st[:], x_t[:])
            x_sb = pp.tile([64, DIM], BF16, tag="x_sb")
            nc.vector.tensor_copy(x_sb[:], x_b[:])

            wc_t = ppp.tile([128, DIM], F32, tag="wc_t")
            wc_b = ppp.tile([64, DIM], F32, tag="wc_b")
            brow = ppp.tile([1, DIM], F32, tag="brow")
            nc.tensor.matmul(brow[:], bv_t, x_st[:], start=True, stop=False)
            nc.tensor.matmul(brow[:], bv_b, x_sb[:], start=False, stop=True)
            nc.tensor.matmul(wc_t[:], wv_t[:, 0:128], x_st[:],
                             start=True, stop=False)
            nc.tensor.matmul(wc_t[:], wv_b[:, 0:128], x_sb[:],
                             start=False, stop=True)
            nc.tensor.matmul(wc_b[:], wv_t[:, 128:DIM], x_st[:],
                             start=True, stop=False)
            nc.tensor.matmul(wc_b[:], wv_b[:, 128:DIM], x_sb[:],
                             start=False, stop=True)

            bc_r = pp.tile([1, DIM], F32, tag="bc_r")
            nc.vector.tensor_add(bc_r[:], brow[:], bo_r)

            # pass-2 rhs: [Wc^T; bc] split at v-channel 128 (+ones row)
            w2_t = pp.tile([128, DIM], BF16, tag="w2_t")
            nc.scalar.copy(w2_t[:], wc_t[:])
            w2_b = pp.tile([65, DIM], BF16, tag="w2_b")
            nc.scalar.copy(w2_b[0:64, :], wc_b[:])
            nc.scalar.copy(w2_b[64:65, :], bc_r[:])

        # ---------------- pass 2: out^T = [v;1]^T [Wc^T; bc] ----------------
        with tc.tile_pool(name="p2_out", bufs=3) as op_, \
             tc.tile_pool(name="p2_psum", bufs=3, space="PSUM") as opp:
            for t in range(NST):
                ot = op_.tile([128, GPS * 384], BF16, tag="ot")
                for s in range(GPS):
                    g = t * GPS + s
                    vt, vbt = v_tiles[g // GPT]
                    off = (g % GPT) * 256
                    pe = opp.tile([128, DIM], F32, tag="pe")
                    po = opp.tile([128, DIM], F32, tag="po")
                    nc.tensor.matmul(pe[:], vt[:, off:off + 128], w2_t[:],
                                     start=True, stop=False)
                    nc.tensor.matmul(pe[:], vbt[:, off:off + 128], w2_b[:],
                                     start=False, stop=True)
                    nc.tensor.matmul(po[:], vt[:, off + 128:off + 256], w2_t[:],
                                     start=True, stop=False)
                    nc.tensor.matmul(po[:], vbt[:, off + 128:off + 256], w2_b[:],
                                     start=False, stop=True)
                    if s % 2 == 0:
                        nc.scalar.copy(ot[:, s * 384:s * 384 + 192], pe[:])
                        nc.vector.tensor_copy(ot[:, s * 384 + 192:(s + 1) * 384], po[:])
                    else:
                        nc.vector.tensor_copy(ot[:, s * 384:s * 384 + 192], pe[:])
                        nc.scalar.copy(ot[:, s * 384 + 192:(s + 1) * 384], po[:])
                nc.sync.dma_start(out_d[t].transpose([1, 0, 2]), ot[:])

    nc.compile()
    return nc


def _get_nc():
    if "nc" not in _CACHE:
        _CACHE["nc"] = _build()
    return _CACHE["nc"]


def _make_in_maps(inputs):
    import ml_dtypes
    f8 = ml_dtypes.float8_e4m3
    bf = ml_dtypes.bfloat16

    q = np.asarray(inputs["q"], dtype=np.float32)
    k = np.asarray(inputs["k"], dtype=np.float32)
    v = np.asarray(inputs["v"], dtype=np.float32)
    wq = np.asarray(inputs["wq"], dtype=np.float32)
    wk = np.asarray(inputs["wk"], dtype=np.float32)
    wv_ = np.asarray(inputs["wv"], dtype=np.float32)
    wo = np.asarray(inputs["wo"], dtype=np.float32)
    bq = np.asarray(inputs["bq"], dtype=np.float32)
    bk = np.asarray(inputs["bk"], dtype=np.float32)
    bv_ = np.asarray(inputs["bv"], dtype=np.float32)
    bo = np.asarray(inputs["bo"], dtype=np.float32)
    temp = np.asarray(inputs["temperature"], dtype=np.float32).reshape(HEADS)

    wb = np.zeros((C1, 772), dtype=np.float32)
    wb[0, 0:192] = bq
    wb[1:, 0:192] = wq.T
    wb[0, 192:384] = bk
    wb[1:, 192:384] = wk.T
    wb[0:192, 384:576] = wv_
    wb[0:192, 576:768] = wo.T
    wb[0:192, 768] = bv_
    wf = np.zeros((1, 200), dtype=np.float32)
    wf[0, 0:192] = bo
    wf[0, 192:198] = temp

    shared = {
        "wb": np.ascontiguousarray(wb.astype(bf)),
        "wf": wf,
    }
    ones_col = np.ones((HW, 1), dtype=np.float32)
    in_maps = []
    for b in range(B):
        m = dict(shared)
        aq = np.concatenate([ones_col, q[b].reshape(DIM, HW).T], axis=1)
        ak = np.concatenate([ones_col, k[b].reshape(DIM, HW).T], axis=1)
        m["q8"] = np.ascontiguousarray(
            aq.astype(f8).reshape(HW // 16, W1))
        m["k8"] = np.ascontiguousarray(
            ak.astype(f8).reshape(HW // 16, W1))
        # permute pixels [even|odd] per 256-block, append ones row
        vp = v[b].reshape(DIM, NG2, 128, 2).transpose(0, 1, 3, 2)
        vb = np.concatenate(
            [vp.reshape(DIM, HW), np.ones((1, HW), np.float32)], axis=0)
        m["vb"] = np.ascontiguousarray(vb.astype(bf))
        in_maps.append(m)
    return in_maps


def _get_runner():
    """Compile once and cache a sharded-jit runner."""
    if "runner" in _CACHE:
        return _CACHE["runner"]
    import jax
    import jax.numpy as jnp
    from jax.sharding import Mesh, PartitionSpec
    from jax.experimental.shard_map import shard_map
    from concourse import bass2jax, mybir as mb
    from concourse.bass2jax import _bass_exec_p, partition_id_tensor

    bass2jax.install_neuronx_cc_hook()
    nc = _get_nc()

    partition_name = nc.partition_id_tensor.name if nc.partition_id_tensor else None
    in_names, out_names, out_avals = [], [], []
    for alloc in nc.m.functions[0].allocations:
        if not isinstance(alloc, mb.MemoryLocationSet):
            continue
        name = alloc.memorylocations[0].name
        if alloc.kind == "ExternalInput":
            if name != partition_name:
                in_names.append(name)
        elif alloc.kind == "ExternalOutput":
            out_names.append(name)
            out_avals.append(jax.core.ShapedArray(
                tuple(alloc.tensor_shape), mb.dt.np(alloc.dtype)))
    n_params = len(in_names)
    n_outs = len(out_avals)
    all_in_names = tuple(in_names + out_names +
                         ([partition_name] if partition_name else []))

    def _body(*args):
        operands = list(args)
        if partition_name is not None:
            operands.append(partition_id_tensor())
        return tuple(_bass_exec_p.bind(
            *operands,
            out_avals=tuple(out_avals),
            in_names=all_in_names,
            out_names=tuple(out_names),
            lowering_input_output_aliases=(),
            sim_require_finite=True,
            sim_require_nnan=True,
            nc=nc,
        ))

    devices = jax.devices()[:B]
    mesh = Mesh(np.asarray(devices), ("core",))
    in_specs = (PartitionSpec("core"),) * (n_params + n_outs)
    out_specs = (PartitionSpec("core"),) * n_outs
    donate = tuple(range(n_params, n_params + n_outs))
    sharded = jax.jit(
        shard_map(_body, mesh=mesh, in_specs=in_specs, out_specs=out_specs,
                  check_rep=False),
        donate_argnums=donate, keep_unused=True)

    zero_shapes = [(B * a.shape[0], *a.shape[1:]) for a in out_avals]
    zero_dtypes = [a.dtype for a in out_avals]

    def run(in_maps):
        concat_in = [
            np.concatenate([np.asarray(in_maps[c][nm]) for c in range(B)], axis=0)
            for nm in in_names
        ]
        zeros = [jnp.zeros(s, d) for s, d in zip(zero_shapes, zero_dtypes)]
        outs = sharded(*concat_in, *zeros)
        return {
            nm: np.asarray(outs[i]).reshape(B, *out_avals[i].shape)
            for i, nm in enumerate(out_names)
        }

    _CACHE["runner"] = run
    return run


def _prebuild():
    """Compile the NEFF and warm the jit at import time; never break import.
    Uses non-degenerate dummy data so norms stay positive (rsqrt-safe)."""
    try:
        import ml_dtypes
        run = _get_runner()
        zq = np.ones((HW // 16, W1), dtype=ml_dtypes.float8_e4m3)
        zv = np.ones((C1, HW), dtype=ml_dtypes.bfloat16)
        zw = np.full((C1, 772), 0.01, dtype=ml_dtypes.bfloat16)
        zf = np.full((1, 200), 0.01, dtype=np.float32)
        run([{"q8": zq, "k8": zq, "vb": zv, "wb": zw, "wf": zf}
             for _ in range(B)])
    except Exception:
        _CACHE.clear()


def kernel(q, k, v, wq, bq, wk, bk, wv, bv, wo, bo, temperature):
    run = _get_runner()
    in_maps = _make_in_maps(dict(q=q, k=k, v=v, wq=wq, bq=bq, wk=wk, bk=bk,
                                 wv=wv, bv=bv, wo=wo, bo=bo,
                                 temperature=temperature))
    out = run(in_maps)["out"]  # (B, NST, GPS, 128, 384) bf16
    out = out.astype(np.float32).reshape(B, HW, DIM)
    out = np.ascontiguousarray(out.transpose(0, 2, 1)).reshape(B, DIM, 128, 128)
    return out


import os as _os
if not _os.environ.get("KERNEL_NO_PREBUILD"):
    _prebuild()
